# revision 8
# baseline (speedup 1.0000x reference)
"""Trainium2 Bass kernel for nn_Kernel_Conv (conv3x3+GELU -> per-pixel 19x19
conv -> conv3x3+sigmoid), SPMD over 8 NeuronCores.

Sharding: 8 cores = 2 batches x 4 H-slices (32 output rows each). All inputs
are host-preprocessed into per-core slabs (bf16) so the device program is
identical on every core.

Structure (v2):
- conv1 emitted transposed: per output row y, out[w, c] = sum_k inp6[k, w] *
  w1[k, c] with k = (dy,dx,ci)+bias-row, so GELU lands directly in the
  xT[w, (row,c)] layout the per-pixel stage needs (no PE transpose phase).
  dy=0,1 are fused into one k=98 matmul via a row-shifted copy of the input.
- per-pixel conv: banded (Toeplitz) matmuls out[c,w] += xT[w',c]*M[w',w],
  4 output rows concurrently via PE column tiling. Band values arrive via
  rectangular DMAs from a host-skewed compact layout ordered so each
  partition's HBM read is one contiguous run.
- conv2 is column-tiled into PSUM quadrants and interleaved into the
  per-pixel loop (conv2 block b emitted after per-pixel block b+1).
"""

import sys
import types

for _p in ("/opt/trn_rl_repo",):
    if _p not in sys.path:
        sys.path.insert(0, _p)

import numpy as np
import ml_dtypes
from contextlib import ExitStack

# Register the NTFF profile hook shim (harmless if tracing is never used)
try:
    import antenv  # noqa: F401
    if "antenv.axon_hooks" not in sys.modules:
        if "/root/.axon_site" not in sys.path:
            sys.path.insert(0, "/root/.axon_site")
        from trn_agent_boot.trn_boot import _ntff_profile_via_ctypes
        _hook = _ntff_profile_via_ctypes("/opt/axon/libaxon_pjrt.so")
        _mod = types.ModuleType("antenv.axon_hooks")
        _mod.get_axon_ntff_profile_hook = lambda: _hook
        sys.modules["antenv.axon_hooks"] = _mod
        antenv.axon_hooks = _mod
except Exception:
    pass

import concourse.bass as bass
import concourse.tile as tile
from concourse import bacc, mybir
from concourse.bass_utils import run_bass_kernel_spmd

BF16 = np.float16

# ---------------- problem constants (hardcoded per the harness contract) ----
B, C, H, W = 2, 16, 128, 128
KK = 19            # per-pixel kernel size
RR = KK // 2       # 9
NCORES = 8
HS = 32            # output rows per core
NY = 36            # y rows per core: [h0-2, h0+34)
NX = 56            # x (conv1 out) rows per core: [h0-11, h0+45)
NIN = 58           # input rows per core: [h0-12, h0+46)
G = 16             # band DMA partition-group size
GW = G + KK - 1    # 34: skewed row width
NG = W // G        # 8 groups
AW = 146           # arena tile width: [-9, 137) -> matmul reads cols [9,137)
NBLK = NY // 4     # 9 per-pixel blocks of 4 y rows
SLOT = KK * AW     # arena free els per y row (19 tiles of AW)
CHUNK_W = 4 * SLOT # arena free els per 4-row chunk
NBUF = 4           # band arena ring depth


def _host_prepare(input, kernel, w1, b1, w2, b2):
    """Build the per-core input maps (all numpy, bf16 except b2)."""
    inp = np.asarray(input, np.float32)
    ker = np.asarray(kernel, np.float32)

    # input, zero-padded: rows [-12, 140), cols [-1, 129)
    inp_pad = np.zeros((B, C, H + 26, W + 2), np.float32)
    inp_pad[:, :, 12:12 + H, 1:1 + W] = inp

    # ker padded along h: rows [-2, 130)
    ker_pad = np.zeros((B, KK * KK, H + 4, W), np.float32)
    ker_pad[:, :, 2:2 + H, :] = ker

    # conv weights as [(dx,c), o] per dy
    def wdy(wmat, dy, order=(0, 1, 2)):
        out = np.zeros((48, 16), np.float32)
        for gi, dx in enumerate(order):
            out[gi * 16:(gi + 1) * 16] = wmat[:, :, dy, dx].T
        return out

    w1f = np.asarray(w1, np.float32)
    # w1s2: col block 0 = mm1 rhs (k=98: dy0 rows 0-47, bias row 48,
    # dy1 rows 49-96, zero row 97); col block 1 rows 0-48 = mm2 rhs (dy2).
    w1s2 = np.zeros((98, 32), np.float32)
    w1s2[0:48, 0:16] = wdy(w1f, 0)
    w1s2[48, 0:16] = np.asarray(b1, np.float32)
    w1s2[49:97, 0:16] = wdy(w1f, 1)
    w1s2[0:48, 16:32] = wdy(w1f, 2)
    w1s2 = np.ascontiguousarray(w1s2).astype(BF16)

    w2f = np.asarray(w2, np.float32)
    w2s = np.zeros((48, 48), np.float32)
    for dy in range(3):
        w2s[:, dy * 16:(dy + 1) * 16] = wdy(w2f, dy, order=(1, 0, 2))
    w2s = np.ascontiguousarray(w2s).astype(BF16)

    # conv2 bias replicated into each PSUM quadrant's partition range
    b2r = np.zeros((128, 1), np.float32)
    for q in range(4):
        b2r[32 * q:32 * q + 16, 0] = np.asarray(b2, np.float32)

    # band data F[yg, kh, g, v, j]:
    #   yr = h0-2+yg; p = G*g+v; w = G*g + j - 9; kw = 18-(j-v); xr = yr+kh-9
    #   F = ker[b, kh*19+kw, yr, w] if all of kw in [0,19), w in [0,W),
    #       yr in [0,H), xr in [0,H) else 0
    yg_i = np.arange(NY)[:, None, None, None, None]
    kh_i = np.arange(KK)[None, :, None, None, None]
    g_i = np.arange(NG)[None, None, :, None, None]
    v_i = np.arange(G)[None, None, None, :, None]
    j_i = np.arange(GW)[None, None, None, None, :]
    kw_i = 18 - (j_i - v_i)
    w_i = G * g_i + j_i - 9
    valid = (kw_i >= 0) & (kw_i < KK) & (w_i >= 0) & (w_i < W)
    kw_c = np.clip(kw_i, 0, KK - 1)
    w_c = np.clip(w_i, 0, W - 1)

    in_maps = []
    for cid in range(NCORES):
        b = cid // 4
        h0 = 32 * (cid % 4)

        # dx-tripled input slab + ones bias row: [49, NIN, W]; inp6 stacks a
        # row-shifted copy on partitions 49-97 so dy=0,1 fuse into one matmul.
        inp3 = np.zeros((49, NIN, W), np.float32)
        rows = inp_pad[b, :, h0: h0 + NIN, :]  # [C, NIN, W+2]
        for dx in range(3):
            inp3[dx * 16:dx * 16 + 16, :, :] = rows[:, :, dx:dx + W]
        inp3[48] = 1.0
        inp6 = np.zeros((98, NIN, W), np.float32)
        inp6[0:49] = inp3
        inp6[49:98, :NIN - 1] = inp3[:, 1:]
        inp6 = inp6.astype(BF16)

        # band F for this core
        yr_i = h0 - 2 + yg_i                 # global y row
        xr_i = yr_i + kh_i - 9               # global x row feeding this tap
        v_ok = valid & (yr_i >= 0) & (yr_i < H) & (xr_i >= 0) & (xr_i < H)
        yr_c = np.clip(yr_i, 0, H - 1)
        p_i = kh_i * KK + kw_c
        F = ker_pad[b, :, 2:2 + H, :][p_i, yr_c, w_c] * v_ok
        # [yg,kh,g,v,j] -> [blk,g,v,r,kh,j]: per (g,v) partition the whole
        # 4-row chunk is contiguous, so the DMA reads HBM sequentially.
        F2 = F.reshape(NBLK, 4, KK, NG, G, GW).transpose(0, 3, 4, 1, 2, 5)
        bandF = np.ascontiguousarray(F2, dtype=BF16)

        in_maps.append({
            "inp6": np.ascontiguousarray(inp6.reshape(98, NIN * W)),
            "bandF": bandF.reshape(NY * KK * NG * G * GW),
            "w1s": w1s2,
            "w2s": w2s,
            "b2r": b2r,
        })
    return in_maps


def _build_program():
    nc = bacc.Bacc("TRN2", target_bir_lowering=False, debug=False,
                   num_devices=NCORES)
    dt = mybir.dt

    inp6_d = nc.dram_tensor("inp6", [98, NIN * W], dt.float16,
                            kind="ExternalInput").ap()
    bandF_d = nc.dram_tensor("bandF", [NY * KK * NG * G * GW], dt.float16,
                             kind="ExternalInput").ap()
    w1s_d = nc.dram_tensor("w1s", [98, 32], dt.float16,
                           kind="ExternalInput").ap()
    w2s_d = nc.dram_tensor("w2s", [48, 48], dt.float16,
                           kind="ExternalInput").ap()
    b2r_d = nc.dram_tensor("b2r", [128, 1], dt.float32,
                           kind="ExternalInput").ap()
    out_d = nc.dram_tensor("out", [16, HS * W], dt.float32,
                           kind="ExternalOutput").ap()

    with tile.TileContext(nc) as tc:
        with ExitStack() as ctx:
            _body(ctx, tc, inp6_d, bandF_d, w1s_d, w2s_d, b2r_d, out_d)
    nc.compile()
    return nc


def _body(ctx, tc, inp6_d, bandF_d, w1s_d, w2s_d, b2r_d, out_d):
    nc = tc.nc
    dt = mybir.dt
    AFT = mybir.ActivationFunctionType

    consts = ctx.enter_context(tc.tile_pool(name="consts", bufs=1))
    bigs = ctx.enter_context(tc.tile_pool(name="bigs", bufs=1))
    outp = ctx.enter_context(tc.tile_pool(name="outp", bufs=2))
    ps_c1 = ctx.enter_context(tc.tile_pool(name="ps_c1", bufs=2, space="PSUM"))
    ps_pp = ctx.enter_context(tc.tile_pool(name="ps_pp", bufs=3, space="PSUM"))
    ps_c2 = ctx.enter_context(tc.tile_pool(name="ps_c2", bufs=2, space="PSUM"))

    # ---- persistent SBUF tiles -------------------------------------------
    w1s_t = consts.tile([98, 32], dt.float16, tag="w1s")
    w2s_t = consts.tile([48, 48], dt.float16, tag="w2s")
    b2r_t = consts.tile([128, 1], dt.float32, tag="b2r")
    inp6_t = bigs.tile([98, NIN * W], dt.float16, tag="inp6")
    xT_t = bigs.tile([128, NX * 16], dt.float16, tag="xT")
    y3_t = bigs.tile([48, NY * W], dt.float16, tag="y3")
    band = [bigs.tile([128, CHUNK_W], dt.float16, tag=f"band{i}",
                      name=f"band{i}") for i in range(NBUF)]

    y3_v = y3_t[:].rearrange("p (r w) -> p r w", r=NY)

    # ---- loads + one-time zeroing ----------------------------------------
    nc.scalar.dma_start(w1s_t[:], w1s_d)
    nc.scalar.dma_start(w2s_t[:], w2s_d)
    nc.scalar.dma_start(b2r_t[:], b2r_d)
    nc.sync.dma_start(inp6_t[0:49, :], inp6_d[0:49, :])
    nc.scalar.dma_start(inp6_t[49:98, :], inp6_d[49:98, :])
    # y3 zero edges (dx-shift copies never write col 0 / col W-1)
    nc.vector.memset(y3_v[16:32, :, 0], 0.0)
    nc.vector.memset(y3_v[32:48, :, 127], 0.0)

    bandF_v = bandF_d.rearrange("(blk g v r kh j) -> blk g v r kh j",
                                blk=NBLK, g=NG, v=G, r=4, kh=KK, j=GW)

    dma_engines = [nc.sync, nc.gpsimd, nc.scalar, nc.sync,
                   nc.gpsimd, nc.scalar, nc.sync, nc.gpsimd]

    def band_chunk_dma(blk):
        dst = band[blk % NBUF]
        for g in range(NG):
            d = dst[G * g: G * g + G, :].rearrange(
                "p (r kh c) -> p r kh c", r=4, kh=KK)[:, :, :, G * g: G * g + GW]
            s = bandF_v[blk, g]
            dma_engines[g].dma_start(d, s)

    HALF = 6716  # vector memsets a bit more than half (it is faster)
    for pre in range(NBUF):
        nc.vector.memset(band[pre][:, :HALF], 0.0)
        nc.gpsimd.memset(band[pre][:, HALF:], 0.0)
        band_chunk_dma(pre)

    # ---- conv1 + GELU, transposed: xT[w, (row,c)] ------------------------
    inp6_v = inp6_t[:].rearrange("p (r w) -> p r w", r=NIN)
    for tb in range(NX // 8):
        ps = ps_c1.tile([128, 128], dt.float32, tag="c1")
        for rr in range(8):
            i = 8 * tb + rr
            nc.tensor.matmul(ps[:, 16 * rr:16 * rr + 16],
                             inp6_v[:, i, :], w1s_t[:, 0:16],
                             start=True, stop=False)
            nc.tensor.matmul(ps[:, 16 * rr:16 * rr + 16],
                             inp6_v[0:49, i + 2, :], w1s_t[0:49, 16:32],
                             start=False, stop=True)
        nc.scalar.activation(xT_t[:, 128 * tb:128 * (tb + 1)], ps[:],
                             AFT.Gelu)

    # ---- conv2 (column-tiled, interleaved below) -------------------------
    c2ps = [None]

    def emit_conv2(b):
        q = b % 4
        if q == 0:
            c2ps[0] = ps_c2.tile([128, 512], dt.float32, tag="c2")
        ps = c2ps[0]
        for dy in range(3):
            nc.tensor.matmul(
                ps[32 * q:32 * q + 16, :],
                w2s_t[:, dy * 16:(dy + 1) * 16],
                y3_v[:, 4 * b + 1 + dy: 4 * b + 5 + dy, :],
                start=(dy == 0), stop=(dy == 2),
                tile_position=(0, 32 * q))
        ot = outp.tile([128, 512], dt.float32, tag="o")
        nc.scalar.activation(ot[32 * q:32 * q + 16, :],
                             ps[32 * q:32 * q + 16, :],
                             AFT.Sigmoid, bias=b2r_t[32 * q:32 * q + 16, :])
        (nc.sync if b % 2 else nc.gpsimd).dma_start(
            out_d[:, 512 * b:512 * (b + 1)], ot[32 * q:32 * q + 16, :])

    # ---- per-pixel conv: banded matmuls, 4-way column tiling -------------
    for blk in range(NBLK):
        bt = band[blk % NBUF]
        pp = ps_pp.tile([128, 128], dt.float32, tag="pp")
        for t in range(22):
            xg = 4 * blk + t
            lhs = xT_t[:, 16 * xg: 16 * xg + 16]
            for g in range(max(0, t - 18), min(3, t) + 1):
                kh = t - g
                off = (g * KK + kh) * AW + 9
                nc.tensor.matmul(
                    pp[32 * g: 32 * g + 16, :],
                    lhs,
                    bt[:, off: off + 128],
                    start=(kh == 0), stop=(kh == KK - 1),
                    tile_position=(0, 32 * g))
        # evict 4 y rows (f32 psum -> bf16 y3, dx=1 slot)
        for g in range(4):
            yg = 4 * blk + g
            nc.vector.tensor_copy(y3_t[0:16, W * yg: W * (yg + 1)],
                                  pp[32 * g: 32 * g + 16, :])
        if blk + NBUF < NBLK:
            band_chunk_dma(blk + NBUF)
        # dx shifts for conv2 (w edges stay zero from the one-time memset)
        nc.scalar.dma_start(y3_v[16:32, 4 * blk: 4 * blk + 4, 1:W],
                            y3_v[0:16, 4 * blk: 4 * blk + 4, 0:W - 1])
        nc.sync.dma_start(y3_v[32:48, 4 * blk: 4 * blk + 4, 0:W - 1],
                          y3_v[0:16, 4 * blk: 4 * blk + 4, 1:W])
        if blk >= 1:
            emit_conv2(blk - 1)


_NC_CACHE = None
LAST = {}


def _get_nc():
    global _NC_CACHE
    if _NC_CACHE is None:
        _NC_CACHE = _build_program()
    return _NC_CACHE


def kernel(input, kernel, w1, b1, w2, b2, _trace=False, _tmpdir=None):
    in_maps = _host_prepare(input, kernel, w1, b1, w2, b2)
    nc = _get_nc()
    res = run_bass_kernel_spmd(nc, in_maps, core_ids=list(range(NCORES)),
                               trace=_trace, tmpdir=_tmpdir)
    out = np.zeros((B, C, H, W), np.float32)
    for cid in range(NCORES):
        b = cid // 4
        h0 = 32 * (cid % 4)
        out[b, :, h0:h0 + HS, :] = res.results[cid]["out"].reshape(16, HS, W)
    LAST["exec_ns"] = res.exec_time_ns
    LAST["trace"] = res.instructions_and_trace
    return out
